# revision 1
# baseline (speedup 1.0000x reference)
"""Trainium2 Bass kernel for nn_Interactor (attention-augmented LSTM).

Problem: B=64, T=512, L=48, DV=DS=H=512.
  per step t: proj_V = x_t W_V^T; proj_R = h W_R^T
              e = tanh(proj_R[:,None,:] + proj_S + proj_V[:,None,:])
              alpha = softmax(e @ w, axis=L); h_ts = alpha @ h_s
              gates = [x_t, h_ts] W_ih^T + h W_hh^T + b; LSTM cell update.

Sharding: data-parallel over batch: 8 cores x 8 batch rows, weights replicated.

Per-core layout ("transposed everything"): feature dims live on SBUF
partitions, the 8 local batch rows on the free dim. State h^T is [512(4x128
chunks), 8]. Recurrent matmuls keep weights stationary (lhsT) and stream the
[128, 8] state as the moving operand. x_t-dependent projections are
precomputed with large matmuls (N=512) into DRAM and streamed back per
T-chunk. Attention softmax runs on a [1, 384] row; the alpha broadcast
across partitions is a single K=1 ones-matmul; the L-reduction is a DVE
blocked tensor_reduce.
"""

import numpy as np

import concourse.bass as bass
import concourse.mybir as mybir
import concourse.tile as tile
from concourse import bacc
from concourse.bass_utils import run_bass_kernel_spmd

F32 = mybir.dt.float32
AF = mybir.ActivationFunctionType
ALU = mybir.AluOpType
AX = mybir.AxisListType

B, T_FULL, L = 64, 512, 48
DV, DS, H = 512, 512, 512
G4 = 4 * H
NCORES = 8
BLOC = B // NCORES  # 8
BL = BLOC * L       # 384
KH = H // 128       # 4 H-chunks
KM = G4 // 128      # 16 gate-row chunks


def build_nc(T=T_FULL, Tc=16, bcast0=True, debug=False):
    """Build the per-core Bass program (SPMD; same program all cores)."""
    assert T % Tc == 0
    nc = bacc.Bacc()

    # ---- DRAM I/O (per-core slices fed via in_maps) ----
    hvT = nc.declare_dram_parameter("hvT", [DV, T * BLOC], F32, isOutput=False)
    hsT = nc.declare_dram_parameter("hsT", [DS, BL], F32, isOutput=False)
    WS_T = nc.declare_dram_parameter("WS_T", [DS, H], F32, isOutput=False)
    WV_T = nc.declare_dram_parameter("WV_T", [DV, H], F32, isOutput=False)
    WihV_T = nc.declare_dram_parameter("WihV_T", [DV, G4], F32, isOutput=False)
    WihS_T = nc.declare_dram_parameter("WihS_T", [DS, G4], F32, isOutput=False)
    Whh_T = nc.declare_dram_parameter("Whh_T", [H, G4], F32, isOutput=False)
    WR_T = nc.declare_dram_parameter("WR_T", [H, H], F32, isOutput=False)
    wvec = nc.declare_dram_parameter("wvec", [H, 1], F32, isOutput=False)
    biasRSV = nc.declare_dram_parameter("biasRSV", [128, KH], F32, isOutput=False)
    biasIH = nc.declare_dram_parameter("biasIH", [128, KM], F32, isOutput=False)
    bw = nc.declare_dram_parameter("bw", [1, 1], F32, isOutput=False)
    out_c = nc.declare_dram_parameter("out_c", [T, KH, 128, BLOC], F32, isOutput=True)

    if debug:
        dbg_rvt = nc.dram_tensor("dbg_rvt", [128, KH * BLOC], F32, kind="ExternalOutput")
        dbg_e = nc.dram_tensor("dbg_e", [128, KH, BL], F32, kind="ExternalOutput")
        dbg_expb = nc.dram_tensor("dbg_expb", [1, BL], F32, kind="ExternalOutput")
        dbg_alpha = nc.dram_tensor("dbg_alpha", [1, BL], F32, kind="ExternalOutput")
        dbg_hts = nc.dram_tensor("dbg_hts", [128, KH * BLOC], F32, kind="ExternalOutput")
        dbg_gates = nc.dram_tensor("dbg_gates", [128, KM * BLOC], F32, kind="ExternalOutput")

    # ---- internal DRAM for precomputed projections ----
    GV_d = nc.dram_tensor("GV_d", [T, KM, 128, BLOC], F32)
    PV_d = nc.dram_tensor("PV_d", [T, KH, 128, BLOC], F32)

    NT = T * BLOC  # hvT free size
    NCW = min(512, NT)  # precompute N-chunk width
    n_nc = NT // NCW

    with tile.TileContext(nc) as tc:
        with (
            tc.tile_pool(name="res", bufs=1) as res,       # resident weights etc
            tc.tile_pool(name="state", bufs=2) as state,   # c state ping-pong
            tc.tile_pool(name="hout", bufs=2) as houtp,    # h ring (8 steps)
            tc.tile_pool(name="stream", bufs=2) as stream, # GV/PV chunks
            tc.tile_pool(name="work", bufs=2) as work,     # per-step tiles
        ):
            # ---------- resident loads ----------
            wr_sb = res.tile([128, KH, H], F32, tag="wr")
            nc.sync.dma_start(out=wr_sb, in_=WR_T.rearrange("(k p) m -> p k m", p=128))
            wihs_sb = res.tile([128, KH, G4], F32, tag="wihs")
            nc.sync.dma_start(out=wihs_sb, in_=WihS_T.rearrange("(k p) m -> p k m", p=128))
            whh_sb = res.tile([128, KH, G4], F32, tag="whh")
            nc.sync.dma_start(out=whh_sb, in_=Whh_T.rearrange("(k p) m -> p k m", p=128))
            hs_sb = res.tile([128, KH, BL], F32, tag="hs")
            nc.sync.dma_start(out=hs_sb, in_=hsT.rearrange("(k p) n -> p k n", p=128))
            wvec_sb = res.tile([128, KH], F32, tag="wvec")
            nc.sync.dma_start(out=wvec_sb, in_=wvec.rearrange("(k p) o -> p (k o)", p=128))
            brsv_sb = res.tile([128, KH], F32, tag="brsv")
            nc.sync.dma_start(out=brsv_sb, in_=biasRSV[:, :])
            bih_sb = res.tile([128, KM], F32, tag="bih")
            nc.sync.dma_start(out=bih_sb, in_=biasIH[:, :])
            bw_sb = res.tile([1, 1], F32, tag="bw")
            nc.sync.dma_start(out=bw_sb, in_=bw[:, :])
            ones_sb = res.tile([1, 128], F32, tag="ones")
            nc.vector.memset(ones_sb, 1.0)
            # PS: proj_S + (b_S+b_R+b_V), [128, KH, BL]
            ps_sb = res.tile([128, KH, BL], F32, tag="ps")

            # ---------- precompute phase ----------
            with (
                tc.tile_pool(name="prew", bufs=1) as prew,
                tc.tile_pool(name="prehv", bufs=4) as prehv,
                tc.tile_pool(name="prestg", bufs=2) as prestg,
                tc.tile_pool(name="prepsum", bufs=4, space="PSUM") as prepsum,
            ):
                ws_sb = prew.tile([128, KH, H], F32, tag="ws")
                nc.sync.dma_start(out=ws_sb, in_=WS_T.rearrange("(k p) m -> p k m", p=128))
                wv_sb = prew.tile([128, KH, H], F32, tag="wv")
                nc.sync.dma_start(out=wv_sb, in_=WV_T.rearrange("(k p) m -> p k m", p=128))
                wihv_sb = prew.tile([128, KH, G4], F32, tag="wihv")
                nc.sync.dma_start(out=wihv_sb, in_=WihV_T.rearrange("(k p) m -> p k m", p=128))

                # PS = W_S @ hsT + biasRSV
                for m in range(KH):
                    pps = prepsum.tile([128, 512], F32, tag="pp")
                    for kc in range(KH):
                        nc.tensor.matmul(
                            pps[:, :BL],
                            ws_sb[:, kc, m * 128:(m + 1) * 128],
                            hs_sb[:, kc, :],
                            start=(kc == 0), stop=(kc == KH - 1),
                        )
                    nc.vector.tensor_scalar_add(ps_sb[:, m, :], pps[:, :BL], brsv_sb[:, m:m + 1])

                # PV / GV over hvT N-chunks of 512
                for ncnk in range(n_nc):
                    nsl = slice(ncnk * NCW, (ncnk + 1) * NCW)
                    hv_t = []
                    for kc in range(KH):
                        t_ = prehv.tile([128, NCW], F32, tag="hv")
                        nc.sync.dma_start(
                            out=t_, in_=hvT[kc * 128:(kc + 1) * 128, nsl])
                        hv_t.append(t_)
                    # PV chunks: out rows m*128, cols (t,b)
                    for m in range(KH):
                        ppv = prepsum.tile([128, NCW], F32, tag="pp")
                        for kc in range(KH):
                            nc.tensor.matmul(
                                ppv, wv_sb[:, kc, m * 128:(m + 1) * 128],
                                hv_t[kc], start=(kc == 0), stop=(kc == KH - 1))
                        stg = prestg.tile([128, NCW], F32, tag="pvstg")
                        nc.scalar.copy(stg, ppv)
                        # dest: PV_d[t0:t0+tw, m, :, :]  dims [t,p,b] <- src [p,(t,b)]
                        t0 = ncnk * NCW // BLOC
                        tw = NCW // BLOC
                        nc.sync.dma_start(
                            out=PV_d[t0:t0 + tw, m, :, :].rearrange("t p b -> p t b"),
                            in_=stg.rearrange("p (t b) -> p t b", b=BLOC))
                    # GV chunks (+ biasIH fold)
                    for m in range(KM):
                        pgv = prepsum.tile([128, NCW], F32, tag="pp")
                        for kc in range(KH):
                            nc.tensor.matmul(
                                pgv, wihv_sb[:, kc, m * 128:(m + 1) * 128],
                                hv_t[kc], start=(kc == 0), stop=(kc == KH - 1))
                        stg = prestg.tile([128, NCW], F32, tag="gvstg")
                        nc.vector.tensor_scalar_add(stg, pgv, bih_sb[:, m:m + 1])
                        t0 = ncnk * NCW // BLOC
                        tw = NCW // BLOC
                        nc.sync.dma_start(
                            out=GV_d[t0:t0 + tw, m, :, :].rearrange("t p b -> p t b"),
                            in_=stg.rearrange("p (t b) -> p t b", b=BLOC))

            # ---------- recurrence ----------
            psum = tc.alloc_tile_pool(name="psum", bufs=2, space="PSUM")
            czero = state.tile([128, 32], F32, tag="c")
            nc.vector.memset(czero, 0.0)
            hzero = res.tile([128, 32], F32, tag="h0")
            nc.vector.memset(hzero, 0.0)
            c_prev = czero
            h_prev = hzero  # [128, (kc,b)]

            HB = 8  # h ring steps per DMA
            gv_cur = pv_cur = None
            hbuf = None

            for t in range(T):
                ic = t % Tc
                if ic == 0:
                    gv_cur = stream.tile([128, Tc, KM, BLOC], F32, tag="gv")
                    nc.sync.dma_start(
                        out=gv_cur,
                        in_=GV_d[t:t + Tc].rearrange("t m p b -> p t m b"))
                    pv_cur = stream.tile([128, Tc, KH, BLOC], F32, tag="pv")
                    nc.sync.dma_start(
                        out=pv_cur,
                        in_=PV_d[t:t + Tc].rearrange("t k p b -> p t k b"))
                ts_ = t % HB
                if ts_ == 0:
                    hbuf = houtp.tile([128, HB, KH, BLOC], F32, tag="hb")

                # 1. proj_R -> psum_rv [128, (kc,b)]
                psum_rv = psum.tile([128, KH, BLOC], F32, tag="rv")
                for m in range(KH):
                    for kc in range(KH):
                        nc.tensor.matmul(
                            psum_rv[:, m, :],
                            wr_sb[:, kc, m * 128:(m + 1) * 128],
                            h_prev[:, kc * BLOC:(kc + 1) * BLOC],
                            start=(kc == 0), stop=(kc == KH - 1))
                # 2. rvt = psum_rv + PV[t]
                rvt = work.tile([128, KH * BLOC], F32, tag="rvt")
                nc.vector.tensor_tensor(
                    rvt.rearrange("p (k b) -> p k b", b=BLOC),
                    psum_rv, pv_cur[:, ic], ALU.add)
                # 3. e = tanh(PS + bcast_L(rvt))  [128, KH, BL]
                e_all = work.tile([128, KH, BL], F32, tag="e")
                for kc in range(KH):
                    if bcast0:
                        sl = rvt[:, kc * BLOC:(kc + 1) * BLOC]
                        bc = bass.AP(tensor=sl.tensor, offset=sl.offset,
                                     ap=[sl.ap[0], [1, BLOC], [0, L]])
                        nc.vector.tensor_tensor(
                            e_all[:, kc].rearrange("p (b l) -> p b l", l=L),
                            hs_ps_view(ps_sb, kc),
                            bc, ALU.add)
                    else:
                        for b in range(BLOC):
                            nc.vector.tensor_scalar_add(
                                e_all[:, kc, b * L:(b + 1) * L],
                                ps_sb[:, kc, b * L:(b + 1) * L],
                                rvt[:, kc * BLOC + b:kc * BLOC + b + 1])
                    nc.scalar.activation(e_all[:, kc], e_all[:, kc], AF.Tanh)
                # 4. beta = wvec . e  -> psum [1, BL]
                psum_beta = psum.tile([1, BL], F32, tag="beta")
                for kc in range(KH):
                    nc.tensor.matmul(
                        psum_beta, wvec_sb[:, kc:kc + 1], e_all[:, kc],
                        start=(kc == 0), stop=(kc == KH - 1))
                # 5. softmax over l (48) per b; no max-subtraction needed
                expb = work.tile([1, BL], F32, tag="expb")
                nc.scalar.activation(expb, psum_beta, AF.Exp, bias=bw_sb[:, 0:1])
                denom = work.tile([1, BLOC], F32, tag="denom")
                nc.vector.tensor_reduce(
                    denom, expb.rearrange("p (b l) -> p b l", l=L), AX.X, ALU.add)
                rec = work.tile([1, BLOC], F32, tag="rec")
                nc.vector.reciprocal(rec, denom)
                alpha = work.tile([1, BL], F32, tag="alpha")
                if bcast0:
                    rbc = bass.AP(tensor=rec.tensor, offset=rec.offset,
                                  ap=[rec.ap[0], [1, BLOC], [0, L]])
                    nc.vector.tensor_tensor(
                        alpha.rearrange("p (b l) -> p b l", l=L),
                        expb.rearrange("p (b l) -> p b l", l=L), rbc, ALU.mult)
                else:
                    for b in range(BLOC):
                        nc.vector.tensor_scalar_mul(
                            alpha[:, b * L:(b + 1) * L],
                            expb[:, b * L:(b + 1) * L], rec[:, b:b + 1])
                # 6. alpha_bc via ones-matmul; apply -> h_tsT [128, (kc,b)]
                psum_abc = psum.tile([128, BL], F32, tag="abc")
                nc.tensor.matmul(psum_abc, ones_sb, alpha, start=True, stop=True)
                h_ts = work.tile([128, KH * BLOC], F32, tag="hts")
                for kc in range(KH):
                    prod = work.tile([128, BL], F32, tag="prod")
                    nc.vector.tensor_tensor(prod, psum_abc, hs_sb[:, kc, :], ALU.mult)
                    nc.vector.tensor_reduce(
                        h_ts[:, kc * BLOC:(kc + 1) * BLOC],
                        prod.rearrange("p (b l) -> p b l", l=L), AX.X, ALU.add)
                # 7. gates = WihS @ h_ts + Whh @ h_prev (+GV via DVE)
                psum_g = psum.tile([128, KM, BLOC], F32, tag="g")
                for m in range(KM):
                    for kc in range(KH):
                        nc.tensor.matmul(
                            psum_g[:, m, :],
                            wihs_sb[:, kc, m * 128:(m + 1) * 128],
                            h_ts[:, kc * BLOC:(kc + 1) * BLOC],
                            start=(kc == 0), stop=False)
                    for kc in range(KH):
                        nc.tensor.matmul(
                            psum_g[:, m, :],
                            whh_sb[:, kc, m * 128:(m + 1) * 128],
                            h_prev[:, kc * BLOC:(kc + 1) * BLOC],
                            start=False, stop=(kc == KH - 1))
                gates = work.tile([128, KM * BLOC], F32, tag="gates")
                nc.vector.tensor_tensor(
                    gates.rearrange("p (m b) -> p m b", b=BLOC),
                    psum_g, gv_cur[:, ic], ALU.add)
                # 8. LSTM pointwise; gate m-chunks: 0-3=i, 4-7=f, 8-11=g, 12-15=o
                gi = gates[:, 0:32]
                gf = gates[:, 32:64]
                gg = gates[:, 64:96]
                go = gates[:, 96:128]
                si = work.tile([128, 32], F32, tag="si")
                nc.scalar.activation(si, gi, AF.Sigmoid)
                sf = work.tile([128, 32], F32, tag="sf")
                nc.scalar.activation(sf, gf, AF.Sigmoid)
                tg = work.tile([128, 32], F32, tag="tg")
                nc.scalar.activation(tg, gg, AF.Tanh)
                so = work.tile([128, 32], F32, tag="so")
                nc.scalar.activation(so, go, AF.Sigmoid)
                t1 = work.tile([128, 32], F32, tag="t1")
                nc.vector.tensor_tensor(t1, si, tg, ALU.mult)
                t2 = work.tile([128, 32], F32, tag="t2")
                nc.vector.tensor_tensor(t2, sf, c_prev, ALU.mult)
                c_new = state.tile([128, 32], F32, tag="c")
                nc.vector.tensor_tensor(c_new, t1, t2, ALU.add)
                tc_ = work.tile([128, 32], F32, tag="tc")
                nc.scalar.activation(tc_, c_new, AF.Tanh)
                h_new = hbuf[:, ts_]  # [128, KH, BLOC] view
                nc.vector.tensor_tensor(
                    h_new.rearrange("p k b -> p (k b)"), so, tc_, ALU.mult)
                if debug and t == 0:
                    nc.sync.dma_start(out=dbg_rvt[:, :], in_=rvt)
                    nc.sync.dma_start(out=dbg_e[:, :, :], in_=e_all)
                    nc.sync.dma_start(out=dbg_expb[:, :], in_=expb)
                    nc.sync.dma_start(out=dbg_alpha[:, :], in_=alpha)
                    nc.sync.dma_start(out=dbg_hts[:, :], in_=h_ts)
                    nc.sync.dma_start(out=dbg_gates[:, :], in_=gates)
                c_prev = c_new
                h_prev = h_new.rearrange("p k b -> p (k b)")
                # 9. flush h ring every HB steps
                if ts_ == HB - 1 or t == T - 1:
                    nb = ts_ + 1
                    t0 = t - nb + 1
                    nc.sync.dma_start(
                        out=out_c[t0:t0 + nb].rearrange("t k p b -> p (t k) b"),
                        in_=hbuf[:, :nb].rearrange("p t k b -> p (t k) b"))
            psum.release()
    nc.finalize()
    return nc


def hs_ps_view(ps_sb, kc):
    v = ps_sb[:, kc, :]
    return v.rearrange("p (b l) -> p b l", l=L)


# ---------------- host side ----------------

def prep_core_inputs(h_v, h_s, W, T=T_FULL):
    """Per-core input maps. W: dict of full weight arrays."""
    WS_T = np.ascontiguousarray(W["W_S"].T)
    WV_T = np.ascontiguousarray(W["W_V"].T)
    WihV_T = np.ascontiguousarray(W["W_ih"][:, :DV].T)
    WihS_T = np.ascontiguousarray(W["W_ih"][:, DV:].T)
    Whh_T = np.ascontiguousarray(W["W_hh"].T)
    WR_T = np.ascontiguousarray(W["W_R"].T)
    wvec = np.ascontiguousarray(W["W_w"][0][:, None])
    biasRSV = np.ascontiguousarray(
        (W["b_S"] + W["b_R"] + W["b_V"]).reshape(KH, 128).T)
    biasIH = np.ascontiguousarray((W["b_ih"] + W["b_hh"]).reshape(KM, 128).T)
    bw = np.ascontiguousarray(W["b_w"].reshape(1, 1))
    maps = []
    for c in range(NCORES):
        bs = slice(c * BLOC, (c + 1) * BLOC)
        hvT = np.ascontiguousarray(
            h_v[bs, :T].transpose(2, 1, 0).reshape(DV, T * BLOC))
        hsT = np.ascontiguousarray(
            h_s[bs].transpose(2, 0, 1).reshape(DS, BLOC * L))
        maps.append({
            "hvT": hvT, "hsT": hsT, "WS_T": WS_T, "WV_T": WV_T,
            "WihV_T": WihV_T, "WihS_T": WihS_T, "Whh_T": Whh_T, "WR_T": WR_T,
            "wvec": wvec, "biasRSV": biasRSV, "biasIH": biasIH, "bw": bw,
        })
    return maps


_NC_CACHE = {}


def kernel(**inputs):
    h_v = np.asarray(inputs["h_v"], dtype=np.float32)
    h_s = np.asarray(inputs["h_s"], dtype=np.float32)
    W = {k: np.asarray(v, dtype=np.float32) for k, v in inputs.items()}
    key = "full"
    if key not in _NC_CACHE:
        _NC_CACHE[key] = build_nc(T=T_FULL, Tc=16, bcast0=True)
    nc = _NC_CACHE[key]
    maps = prep_core_inputs(h_v, h_s, W, T=T_FULL)
    res = run_bass_kernel_spmd(nc, maps, list(range(NCORES)))
    outs = []
    for c in range(NCORES):
        arr = res.results[c]["out_c"]  # [T, KH, 128, BLOC]
        outs.append(np.ascontiguousarray(
            arr.transpose(3, 0, 1, 2).reshape(BLOC, T_FULL, H)))
    return np.concatenate(outs, axis=0).astype(np.float32)


if __name__ == "__main__":
    # smoke build
    nc = build_nc(T=8, Tc=4)
    print("built ok:", len(nc.m.functions[0].instructions) if hasattr(nc.m.functions[0], 'instructions') else "?")



# revision 8
# speedup vs baseline: 4.8091x; 4.8091x over previous
"""Trainium2 Bass kernel for nn_Interactor (attention-augmented LSTM).

Problem: B=64, T=512, L=48, DV=DS=H=512.
  per step t: proj_V = x_t W_V^T; proj_R = h W_R^T
              e = tanh(proj_R[:,None,:] + proj_S + proj_V[:,None,:])
              alpha = softmax(e @ w, axis=L); h_ts = alpha @ h_s
              gates = [x_t, h_ts] W_ih^T + h W_hh^T + b; LSTM cell update.

Sharding: data-parallel over batch: 8 cores x 8 batch rows, weights replicated.

v2 design notes (vs v1):
 - All tensor-engine operands in bf16: fp32 matmuls are emitted as 2 HW
   instructions (HI/LO split) and disable fast-weight-load; bf16 halves
   instruction count and cuts LDWEIGHTS from ~325ns to ~55ns per tile.
 - LSTM sigmoid eliminated via sigma(x) = 0.5*tanh(x/2)+0.5 (i/f/o weight rows
   pre-scaled 0.5 on host). Kernel uses only {tanh, exp} => single ACT table
   set, killing 2x ACT_TABLE_LOAD (~2.7us) per step.
 - h_ts = alpha @ h_s computed on the tensor engine: alpha lands on
   partitions via outer-product matmul expb_chunk^T (x) rec, masked to
   block-diagonal A, then h_s-stationary matmuls produce h_ts^T [D,b].
 - W_hh @ h_prev issued right after proj_R so it overlaps the attention
   chain on DVE/ACT; W_ihS @ h_ts completes the PSUM group later.
 - Gate order remapped to [i, f, o, g] so one batched tanh + one affine
   covers the pointwise nonlinearities.
"""

import numpy as np
import ml_dtypes

import concourse.bass as bass
import concourse.mybir as mybir
import concourse.tile as tile
from concourse import bacc
from concourse.bass_utils import run_bass_kernel_spmd

F32 = mybir.dt.float32
BF16 = mybir.dt.bfloat16
AF = mybir.ActivationFunctionType
ALU = mybir.AluOpType
AX = mybir.AxisListType
NP_BF16 = ml_dtypes.bfloat16

B, T_FULL, L = 64, 512, 48
DV, DS, H = 512, 512, 512
G4 = 4 * H
NCORES = 8
BLOC = B // NCORES  # 8
BL = BLOC * L       # 384
KH = H // 128       # 4 H-chunks
KM = G4 // 128      # 16 gate-row chunks
KA = BL // 128      # 3 (b,l)-chunks


def build_nc(T=T_FULL, Tc=16, debug=False, dbg_t=0):
    """Build the per-core Bass program (SPMD; same program all cores)."""
    assert T % Tc == 0
    nc = bacc.Bacc()

    # ---- DRAM I/O (per-core slices fed via in_maps) ----
    hvT = nc.declare_dram_parameter("hvT", [DV, T * BLOC], BF16, isOutput=False)
    hsT = nc.declare_dram_parameter("hsT", [DS, BL], BF16, isOutput=False)
    hs_part = nc.declare_dram_parameter("hs_part", [BL, DS], BF16, isOutput=False)
    WS_T = nc.declare_dram_parameter("WS_T", [DS, H], BF16, isOutput=False)
    WV_T = nc.declare_dram_parameter("WV_T", [DV, H], BF16, isOutput=False)
    WihV_T = nc.declare_dram_parameter("WihV_T", [DV, G4], BF16, isOutput=False)
    WihS_T = nc.declare_dram_parameter("WihS_T", [DS, G4], BF16, isOutput=False)
    Whh_T = nc.declare_dram_parameter("Whh_T", [H, G4], BF16, isOutput=False)
    WR_T = nc.declare_dram_parameter("WR_T", [H, H], BF16, isOutput=False)
    wvec = nc.declare_dram_parameter("wvec", [H, 1], BF16, isOutput=False)
    maskD = nc.declare_dram_parameter("maskD", [BL, BLOC], F32, isOutput=False)
    biasRSV = nc.declare_dram_parameter("biasRSV", [128, KH], F32, isOutput=False)
    biasIH = nc.declare_dram_parameter("biasIH", [128, KM], F32, isOutput=False)
    bw = nc.declare_dram_parameter("bw", [1, 1], F32, isOutput=False)
    out_c = nc.declare_dram_parameter("out_c", [T, KH, 128, BLOC], BF16, isOutput=True)

    # ---- internal DRAM for precomputed projections ----
    GV_d = nc.dram_tensor("GV_d", [T, KM, 128, BLOC], F32)
    PV_d = nc.dram_tensor("PV_d", [T, KH, 128, BLOC], F32)

    if debug:
        dbg = {
            "dbg_rvt": nc.dram_tensor("dbg_rvt", [128, KH * BLOC], BF16, kind="ExternalOutput"),
            "dbg_e": nc.dram_tensor("dbg_e", [128, KH, BL], BF16, kind="ExternalOutput"),
            "dbg_expb": nc.dram_tensor("dbg_expb", [1, BL], BF16, kind="ExternalOutput"),
            "dbg_A": nc.dram_tensor("dbg_A", [128, KA, BLOC], BF16, kind="ExternalOutput"),
            "dbg_hts": nc.dram_tensor("dbg_hts", [128, KH * BLOC], BF16, kind="ExternalOutput"),
            "dbg_gates": nc.dram_tensor("dbg_gates", [128, KM * BLOC], F32, kind="ExternalOutput"),
            "dbg_th": nc.dram_tensor("dbg_th", [128, KM * BLOC], F32, kind="ExternalOutput"),
            "dbg_c": nc.dram_tensor("dbg_c", [128, 32], F32, kind="ExternalOutput"),
        }

    NT = T * BLOC
    NCW = min(512, NT)
    n_nc = NT // NCW

    with tile.TileContext(nc) as tc:
        with (
            tc.tile_pool(name="res", bufs=1) as res,
            tc.tile_pool(name="state", bufs=2) as state,
            tc.tile_pool(name="hout", bufs=2) as houtp,
            tc.tile_pool(name="stream", bufs=2) as stream,
            tc.tile_pool(name="work", bufs=2) as work,
        ):
            # ---------- resident loads ----------
            wr_sb = res.tile([128, KH, H], BF16, tag="wr")
            nc.sync.dma_start(out=wr_sb, in_=WR_T.rearrange("(k p) m -> p k m", p=128))
            wihs_sb = res.tile([128, KH, G4], BF16, tag="wihs")
            nc.sync.dma_start(out=wihs_sb, in_=WihS_T.rearrange("(k p) m -> p k m", p=128))
            whh_sb = res.tile([128, KH, G4], BF16, tag="whh")
            nc.sync.dma_start(out=whh_sb, in_=Whh_T.rearrange("(k p) m -> p k m", p=128))
            hsp_sb = res.tile([128, KA, DS], BF16, tag="hsp")
            nc.sync.dma_start(out=hsp_sb, in_=hs_part.rearrange("(c p) d -> p c d", p=128))
            wvec_sb = res.tile([128, KH], BF16, tag="wvec")
            nc.sync.dma_start(out=wvec_sb, in_=wvec.rearrange("(k p) o -> p (k o)", p=128))
            mask_sb = res.tile([128, KA, BLOC], F32, tag="mask")
            nc.sync.dma_start(out=mask_sb, in_=maskD.rearrange("(c p) b -> p c b", p=128))
            brsv_sb = res.tile([128, KH], F32, tag="brsv")
            nc.sync.dma_start(out=brsv_sb, in_=biasRSV[:, :])
            bih_sb = res.tile([128, KM], F32, tag="bih")
            nc.sync.dma_start(out=bih_sb, in_=biasIH[:, :])
            bw_sb = res.tile([1, 1], F32, tag="bw")
            nc.sync.dma_start(out=bw_sb, in_=bw[:, :])
            # PS: proj_S + (b_S+b_R+b_V), [128, KH, BL] bf16
            ps_sb = res.tile([128, KH, BL], BF16, tag="ps")

            # ---------- precompute phase ----------
            with (
                tc.tile_pool(name="prew", bufs=1) as prew,
                tc.tile_pool(name="prehv", bufs=4) as prehv,
                tc.tile_pool(name="prestg", bufs=2) as prestg,
                tc.tile_pool(name="prepsum", bufs=4, space="PSUM") as prepsum,
            ):
                ws_sb = prew.tile([128, KH, H], BF16, tag="ws")
                nc.sync.dma_start(out=ws_sb, in_=WS_T.rearrange("(k p) m -> p k m", p=128))
                wv_sb = prew.tile([128, KH, H], BF16, tag="wv")
                nc.sync.dma_start(out=wv_sb, in_=WV_T.rearrange("(k p) m -> p k m", p=128))
                wihv_sb = prew.tile([128, KH, G4], BF16, tag="wihv")
                nc.sync.dma_start(out=wihv_sb, in_=WihV_T.rearrange("(k p) m -> p k m", p=128))
                hsT_sb = prew.tile([128, KH, BL], BF16, tag="hsT")
                nc.sync.dma_start(out=hsT_sb, in_=hsT.rearrange("(k p) n -> p k n", p=128))

                # PS = W_S @ hsT + biasRSV
                for m in range(KH):
                    pps = prepsum.tile([128, BL], F32, tag="pp")
                    for kc in range(KH):
                        nc.tensor.matmul(
                            pps,
                            ws_sb[:, kc, m * 128:(m + 1) * 128],
                            hsT_sb[:, kc, :],
                            start=(kc == 0), stop=(kc == KH - 1),
                        )
                    nc.vector.tensor_scalar_add(ps_sb[:, m, :], pps, brsv_sb[:, m:m + 1])

                # PV / GV over hvT N-chunks
                for ncnk in range(n_nc):
                    nsl = slice(ncnk * NCW, (ncnk + 1) * NCW)
                    hv_t = []
                    for kc in range(KH):
                        t_ = prehv.tile([128, NCW], BF16, tag="hv")
                        nc.sync.dma_start(
                            out=t_, in_=hvT[kc * 128:(kc + 1) * 128, nsl])
                        hv_t.append(t_)
                    t0 = ncnk * NCW // BLOC
                    tw = NCW // BLOC
                    for m in range(KH):
                        ppv = prepsum.tile([128, NCW], F32, tag="pp")
                        for kc in range(KH):
                            nc.tensor.matmul(
                                ppv, wv_sb[:, kc, m * 128:(m + 1) * 128],
                                hv_t[kc], start=(kc == 0), stop=(kc == KH - 1))
                        stg = prestg.tile([128, NCW], F32, tag="pvstg")
                        nc.scalar.copy(stg, ppv)
                        nc.sync.dma_start(
                            out=PV_d[t0:t0 + tw, m, :, :].rearrange("t p b -> p t b"),
                            in_=stg.rearrange("p (t b) -> p t b", b=BLOC))
                    for m in range(KM):
                        pgv = prepsum.tile([128, NCW], F32, tag="pp")
                        for kc in range(KH):
                            nc.tensor.matmul(
                                pgv, wihv_sb[:, kc, m * 128:(m + 1) * 128],
                                hv_t[kc], start=(kc == 0), stop=(kc == KH - 1))
                        stg = prestg.tile([128, NCW], F32, tag="gvstg")
                        nc.vector.tensor_scalar_add(stg, pgv, bih_sb[:, m:m + 1])
                        nc.sync.dma_start(
                            out=GV_d[t0:t0 + tw, m, :, :].rearrange("t p b -> p t b"),
                            in_=stg.rearrange("p (t b) -> p t b", b=BLOC))

            # ---------- recurrence ----------
            psum = tc.alloc_tile_pool(name="psum", bufs=1, space="PSUM")
            czero = state.tile([128, 32], F32, tag="c")
            nc.vector.memset(czero, 0.0)
            hzero = res.tile([128, 32], BF16, tag="h0")
            nc.vector.memset(hzero, 0.0)
            c_prev = czero
            h_prev = hzero  # [128, (kc,b)] bf16

            HB = 8
            gv_cur = pv_cur = None
            hbuf = None

            for t in range(T):
                ic = t % Tc
                if ic == 0:
                    gv_cur = stream.tile([128, Tc, KM, BLOC], F32, tag="gv")
                    nc.sync.dma_start(
                        out=gv_cur,
                        in_=GV_d[t:t + Tc].rearrange("t m p b -> p t m b"))
                    pv_cur = stream.tile([128, Tc, KH, BLOC], F32, tag="pv")
                    nc.sync.dma_start(
                        out=pv_cur,
                        in_=PV_d[t:t + Tc].rearrange("t k p b -> p t k b"))
                ts_ = t % HB
                if ts_ == 0:
                    hbuf = houtp.tile([128, HB, KH, BLOC], BF16, tag="hb")

                # 1. proj_R -> psum_rv [128, (m,b)]
                psum_rv = psum.tile([128, KH, BLOC], F32, tag="rv")
                for m in range(KH):
                    for kc in range(KH):
                        nc.tensor.matmul(
                            psum_rv[:, m, :],
                            wr_sb[:, kc, m * 128:(m + 1) * 128],
                            h_prev[:, kc * BLOC:(kc + 1) * BLOC],
                            start=(kc == 0), stop=(kc == KH - 1))
                # 2. gates: W_hh @ h_prev part first (overlaps attention chain)
                psum_gh = psum.tile([128, KM, BLOC], F32, tag="gh")
                for m in range(KM):
                    for kc in range(KH):
                        nc.tensor.matmul(
                            psum_gh[:, m, :],
                            whh_sb[:, kc, m * 128:(m + 1) * 128],
                            h_prev[:, kc * BLOC:(kc + 1) * BLOC],
                            start=(kc == 0), stop=(kc == KH - 1))
                # 3. rvt = proj_R + PV[t]  (bf16)
                rvt = work.tile([128, KH * BLOC], BF16, tag="rvt")
                nc.vector.tensor_tensor(
                    rvt.rearrange("p (k b) -> p k b", b=BLOC),
                    psum_rv, pv_cur[:, ic], ALU.add)
                # 4. e = tanh(PS + bcast_L(rvt)); beta accumulation
                e_all = work.tile([128, KH, BL], BF16, tag="e")
                psum_beta = psum.tile([1, BL], F32, tag="beta")
                for kc in range(KH):
                    sl = rvt[:, kc * BLOC:(kc + 1) * BLOC]
                    bc = bass.AP(tensor=sl.tensor, offset=sl.offset,
                                 ap=[sl.ap[0], [1, BLOC], [0, L]])
                    nc.vector.tensor_tensor(
                        e_all[:, kc].rearrange("p (b l) -> p b l", l=L),
                        ps_sb[:, kc].rearrange("p (b l) -> p b l", l=L),
                        bc, ALU.add)
                    nc.scalar.activation(e_all[:, kc], e_all[:, kc], AF.Tanh)
                    nc.tensor.matmul(
                        psum_beta, wvec_sb[:, kc:kc + 1], e_all[:, kc],
                        start=(kc == 0), stop=(kc == KH - 1))
                # 5. softmax pieces: expb (bf16), denom, rec
                expb = work.tile([1, BL], BF16, tag="expb")
                nc.scalar.activation(expb, psum_beta, AF.Exp, bias=bw_sb[:, 0:1])
                denom = work.tile([1, BLOC], F32, tag="denom")
                nc.vector.tensor_reduce(
                    denom, expb.rearrange("p (b l) -> p b l", l=L), AX.X, ALU.add)
                rec = work.tile([1, BLOC], F32, tag="rec")
                nc.vector.reciprocal(rec, denom)
                rec_bf = work.tile([1, BLOC], BF16, tag="recbf")
                nc.vector.tensor_copy(rec_bf, rec)
                # 6. alpha onto partitions: psum_A[c] = expb_chunk^T (x) rec
                psum_A = psum.tile([128, KA, BLOC], F32, tag="A")
                for c in range(KA):
                    nc.tensor.matmul(
                        psum_A[:, c, :],
                        expb[:, c * 128:(c + 1) * 128],
                        rec_bf, start=True, stop=True)
                A_sb = work.tile([128, KA, BLOC], BF16, tag="Asb")
                nc.vector.tensor_tensor(A_sb, psum_A, mask_sb, ALU.mult)
                # 7. h_ts^T = hs_part^T @ A  -> [128, (m,b)]
                psum_hts = psum.tile([128, KH, BLOC], F32, tag="hts")
                for m in range(KH):
                    for c in range(KA):
                        nc.tensor.matmul(
                            psum_hts[:, m, :],
                            hsp_sb[:, c, m * 128:(m + 1) * 128],
                            A_sb[:, c, :],
                            start=(c == 0), stop=(c == KA - 1))
                hts_bf = work.tile([128, KH * BLOC], BF16, tag="htsbf")
                nc.scalar.copy(hts_bf.rearrange("p (k b) -> p k b", b=BLOC), psum_hts)
                # 8. gates: W_ihS @ h_ts into its own PSUM tile
                psum_gs = psum.tile([128, KM, BLOC], F32, tag="gs")
                for m in range(KM):
                    for kc in range(KH):
                        nc.tensor.matmul(
                            psum_gs[:, m, :],
                            wihs_sb[:, kc, m * 128:(m + 1) * 128],
                            hts_bf[:, kc * BLOC:(kc + 1) * BLOC],
                            start=(kc == 0), stop=(kc == KH - 1))
                # 9. gates = psum_gh + GV[t] + psum_gs; gate order [i, f, o, g]
                gtmp = work.tile([128, KM * BLOC], F32, tag="gtmp")
                nc.vector.tensor_tensor(
                    gtmp.rearrange("p (m b) -> p m b", b=BLOC),
                    psum_gh, gv_cur[:, ic], ALU.add)
                gates = work.tile([128, KM * BLOC], F32, tag="gates")
                nc.vector.tensor_tensor(
                    gates.rearrange("p (m b) -> p m b", b=BLOC),
                    psum_gs, gtmp.rearrange("p (m b) -> p m b", b=BLOC), ALU.add)
                # 10. pointwise: th = tanh(gates) (i/f/o rows pre-scaled 0.5)
                th = work.tile([128, KM * BLOC], F32, tag="th")
                nc.scalar.activation(th, gates, AF.Tanh)
                sio = work.tile([128, 96], F32, tag="sio")
                nc.vector.tensor_scalar(
                    sio, th[:, 0:96], 0.5, 0.5, ALU.mult, ALU.add)
                t1 = work.tile([128, 32], F32, tag="t1")
                nc.vector.tensor_tensor(t1, sio[:, 0:32], th[:, 96:128], ALU.mult)
                t2 = work.tile([128, 32], F32, tag="t2")
                nc.vector.tensor_tensor(t2, sio[:, 32:64], c_prev, ALU.mult)
                c_new = state.tile([128, 32], F32, tag="c")
                nc.vector.tensor_tensor(c_new, t1, t2, ALU.add)
                tc_ = work.tile([128, 32], F32, tag="tc")
                nc.scalar.activation(tc_, c_new, AF.Tanh)
                h_new = hbuf[:, ts_]  # [128, KH, BLOC] bf16 view
                nc.vector.tensor_tensor(
                    h_new.rearrange("p k b -> p (k b)"), sio[:, 64:96], tc_, ALU.mult)
                if debug and t == dbg_t:
                    nc.sync.dma_start(out=dbg["dbg_rvt"][:, :], in_=rvt)
                    nc.sync.dma_start(out=dbg["dbg_e"][:, :, :], in_=e_all)
                    nc.sync.dma_start(out=dbg["dbg_expb"][:, :], in_=expb)
                    nc.sync.dma_start(out=dbg["dbg_A"][:, :, :], in_=A_sb)
                    nc.sync.dma_start(out=dbg["dbg_hts"][:, :], in_=hts_bf)
                    nc.sync.dma_start(out=dbg["dbg_gates"][:, :], in_=gates)
                    nc.sync.dma_start(out=dbg["dbg_th"][:, :], in_=th)
                    nc.sync.dma_start(out=dbg["dbg_c"][:, :], in_=c_new)
                c_prev = c_new
                h_prev = h_new.rearrange("p k b -> p (k b)")
                # 11. flush h ring
                if ts_ == HB - 1 or t == T - 1:
                    nb = ts_ + 1
                    t0 = t - nb + 1
                    nc.sync.dma_start(
                        out=out_c[t0:t0 + nb].rearrange("t k p b -> p (t k) b"),
                        in_=hbuf[:, :nb].rearrange("p t k b -> p (t k) b"))
            psum.release()
    nc.finalize()
    return nc


# ---------------- host side ----------------

def _gate_remap():
    """Row permutation + scale for gate order [i, f, o, g], i/f/o scaled 0.5."""
    idx = np.concatenate([
        np.arange(0, 512), np.arange(512, 1024),
        np.arange(1536, 2048), np.arange(1024, 1536)])
    scale = np.concatenate([
        np.full(1536, 0.5, np.float32), np.ones(512, np.float32)])
    return idx, scale


def prep_core_inputs(h_v, h_s, W, T=T_FULL):
    """Per-core input maps. W: dict of full weight arrays."""
    idx, gsc = _gate_remap()
    W_ih2 = W["W_ih"][idx] * gsc[:, None]
    W_hh2 = W["W_hh"][idx] * gsc[:, None]
    b2 = (W["b_ih"] + W["b_hh"])[idx] * gsc

    WS_T = np.ascontiguousarray(W["W_S"].T).astype(NP_BF16)
    WV_T = np.ascontiguousarray(W["W_V"].T).astype(NP_BF16)
    WihV_T = np.ascontiguousarray(W_ih2[:, :DV].T).astype(NP_BF16)
    WihS_T = np.ascontiguousarray(W_ih2[:, DV:].T).astype(NP_BF16)
    Whh_T = np.ascontiguousarray(W_hh2.T).astype(NP_BF16)
    WR_T = np.ascontiguousarray(W["W_R"].T).astype(NP_BF16)
    wvec = np.ascontiguousarray(W["W_w"][0][:, None]).astype(NP_BF16)
    biasRSV = np.ascontiguousarray(
        (W["b_S"] + W["b_R"] + W["b_V"]).reshape(KH, 128).T).astype(np.float32)
    biasIH = np.ascontiguousarray(b2.reshape(KM, 128).T).astype(np.float32)
    bw = np.ascontiguousarray(W["b_w"].reshape(1, 1)).astype(np.float32)
    maskD = np.zeros((BL, BLOC), np.float32)
    for j in range(BL):
        maskD[j, j // L] = 1.0
    maps = []
    for c in range(NCORES):
        bs = slice(c * BLOC, (c + 1) * BLOC)
        hvT = np.ascontiguousarray(
            h_v[bs, :T].transpose(2, 1, 0).reshape(DV, T * BLOC)).astype(NP_BF16)
        hsT = np.ascontiguousarray(
            h_s[bs].transpose(2, 0, 1).reshape(DS, BLOC * L)).astype(NP_BF16)
        hs_part = np.ascontiguousarray(
            h_s[bs].reshape(BL, DS)).astype(NP_BF16)
        maps.append({
            "hvT": hvT, "hsT": hsT, "hs_part": hs_part,
            "WS_T": WS_T, "WV_T": WV_T,
            "WihV_T": WihV_T, "WihS_T": WihS_T, "Whh_T": Whh_T, "WR_T": WR_T,
            "wvec": wvec, "maskD": maskD,
            "biasRSV": biasRSV, "biasIH": biasIH, "bw": bw,
        })
    return maps


_NC_CACHE = {}


def kernel(**inputs):
    h_v = np.asarray(inputs["h_v"], dtype=np.float32)
    h_s = np.asarray(inputs["h_s"], dtype=np.float32)
    W = {k: np.asarray(v, dtype=np.float32) for k, v in inputs.items()}
    key = "full"
    if key not in _NC_CACHE:
        _NC_CACHE[key] = build_nc(T=T_FULL, Tc=16)
    nc = _NC_CACHE[key]
    maps = prep_core_inputs(h_v, h_s, W, T=T_FULL)
    res = run_bass_kernel_spmd(nc, maps, list(range(NCORES)))
    outs = []
    for c in range(NCORES):
        arr = np.asarray(res.results[c]["out_c"]).astype(np.float32)
        outs.append(np.ascontiguousarray(
            arr.transpose(3, 0, 1, 2).reshape(BLOC, T_FULL, H)))
    return np.concatenate(outs, axis=0).astype(np.float32)


if __name__ == "__main__":
    nc = build_nc(T=8, Tc=4)
    print("built ok")


# revision 19
# speedup vs baseline: 6.2627x; 1.3022x over previous
"""Trainium2 Bass kernel for nn_Interactor (attention-augmented LSTM).

Problem: B=64, T=512, L=48, DV=DS=H=512.
  per step t: proj_V = x_t W_V^T; proj_R = h W_R^T
              e = tanh(proj_R[:,None,:] + proj_S + proj_V[:,None,:])
              alpha = softmax(e @ w, axis=L); h_ts = alpha @ h_s
              gates = [x_t, h_ts] W_ih^T + h W_hh^T + b; LSTM cell update.

Sharding: data-parallel over batch: 8 cores x 8 batch rows, weights replicated.

v2 design notes (vs v1):
 - All tensor-engine operands in bf16: fp32 matmuls are emitted as 2 HW
   instructions (HI/LO split) and disable fast-weight-load; bf16 halves
   instruction count and cuts LDWEIGHTS from ~325ns to ~55ns per tile.
 - LSTM sigmoid eliminated via sigma(x) = 0.5*tanh(x/2)+0.5 (i/f/o weight rows
   pre-scaled 0.5 on host). Kernel uses only {tanh, exp} => single ACT table
   set, killing 2x ACT_TABLE_LOAD (~2.7us) per step.
 - h_ts = alpha @ h_s computed on the tensor engine: alpha lands on
   partitions via outer-product matmul expb_chunk^T (x) rec, masked to
   block-diagonal A, then h_s-stationary matmuls produce h_ts^T [D,b].
 - W_hh @ h_prev issued right after proj_R so it overlaps the attention
   chain on DVE/ACT; W_ihS @ h_ts completes the PSUM group later.
 - Gate order remapped to [i, f, o, g] so one batched tanh + one affine
   covers the pointwise nonlinearities.
"""

import numpy as np
import ml_dtypes

import concourse.bass as bass
import concourse.mybir as mybir
import concourse.tile as tile
from concourse import bacc
from concourse.bass_utils import run_bass_kernel_spmd

F32 = mybir.dt.float32
BF16 = mybir.dt.bfloat16
AF = mybir.ActivationFunctionType
ALU = mybir.AluOpType
AX = mybir.AxisListType
NP_BF16 = ml_dtypes.bfloat16

B, T_FULL, L = 64, 512, 48
DV, DS, H = 512, 512, 512
G4 = 4 * H
NCORES = 8
BLOC = B // NCORES  # 8
BL = BLOC * L       # 384
KH = H // 128       # 4 H-chunks
KM = G4 // 128      # 16 gate-row chunks
KA = BL // 128      # 3 (b,l)-chunks


def build_nc(T=T_FULL, Tc=16, debug=False, dbg_t=0):
    """Build the per-core Bass program (SPMD; same program all cores)."""
    assert T % Tc == 0
    nc = bacc.Bacc()

    # ---- DRAM I/O (per-core slices fed via in_maps) ----
    hvT = nc.declare_dram_parameter("hvT", [DV, T * BLOC], BF16, isOutput=False)
    hsT = nc.declare_dram_parameter("hsT", [DS, BL], BF16, isOutput=False)
    WS_T = nc.declare_dram_parameter("WS_T", [DS, H], BF16, isOutput=False)
    WV_T = nc.declare_dram_parameter("WV_T", [DV, H], BF16, isOutput=False)
    WihV_T = nc.declare_dram_parameter("WihV_T", [DV, G4], BF16, isOutput=False)
    WihS_T = nc.declare_dram_parameter("WihS_T", [DS, G4], BF16, isOutput=False)
    Whh_T = nc.declare_dram_parameter("Whh_T", [H, G4], BF16, isOutput=False)
    WR_T = nc.declare_dram_parameter("WR_T", [H, H], BF16, isOutput=False)
    wvec = nc.declare_dram_parameter("wvec", [H, 1], BF16, isOutput=False)
    maskD = nc.declare_dram_parameter("maskD", [BL, BLOC], F32, isOutput=False)
    biasRSV = nc.declare_dram_parameter("biasRSV", [128, KH], F32, isOutput=False)
    biasIH = nc.declare_dram_parameter("biasIH", [128, KM], F32, isOutput=False)
    bw = nc.declare_dram_parameter("bw", [1, 1], F32, isOutput=False)
    out_c = nc.declare_dram_parameter("out_c", [T, KH, 128, BLOC], BF16, isOutput=True)

    # ---- internal DRAM for precomputed projections ----
    GV_d = nc.dram_tensor("GV_d", [T, KM, 128, BLOC], F32)
    PV_d = nc.dram_tensor("PV_d", [T, KH, 128, BLOC], F32)

    if debug:
        dbg = {
            "dbg_rvt": nc.dram_tensor("dbg_rvt", [128, KH * BLOC], BF16, kind="ExternalOutput"),
            "dbg_e": nc.dram_tensor("dbg_e", [128, KH, BL], BF16, kind="ExternalOutput"),
            "dbg_expb": nc.dram_tensor("dbg_expb", [1, BL], BF16, kind="ExternalOutput"),
            "dbg_A": nc.dram_tensor("dbg_A", [128, KA, BLOC], BF16, kind="ExternalOutput"),
            "dbg_gates": nc.dram_tensor("dbg_gates", [128, KM * BLOC], F32, kind="ExternalOutput"),
            "dbg_th": nc.dram_tensor("dbg_th", [128, KM * BLOC], F32, kind="ExternalOutput"),
            "dbg_c": nc.dram_tensor("dbg_c", [128, 32], F32, kind="ExternalOutput"),
        }

    NT = T * BLOC
    NCW = min(512, NT)
    n_nc = NT // NCW

    with tile.TileContext(nc) as tc:
        with (
            tc.tile_pool(name="res", bufs=1) as res,
            tc.tile_pool(name="state", bufs=2) as state,
            tc.tile_pool(name="hout", bufs=2) as houtp,
            tc.tile_pool(name="stream", bufs=2) as stream,
            tc.tile_pool(name="work", bufs=2) as work,
        ):
            # ---------- resident loads ----------
            wr_sb = res.tile([128, KH, H], BF16, tag="wr")
            nc.sync.dma_start(out=wr_sb, in_=WR_T.rearrange("(k p) m -> p k m", p=128))
            whh_sb = res.tile([128, KH, G4], BF16, tag="whh")
            nc.sync.dma_start(out=whh_sb, in_=Whh_T.rearrange("(k p) m -> p k m", p=128))
            psg_sb = res.tile([128, KA, G4], BF16, tag="psg")
            wvec_sb = res.tile([128, KH], BF16, tag="wvec")
            nc.sync.dma_start(out=wvec_sb, in_=wvec.rearrange("(k p) o -> p (k o)", p=128))
            mask_sb = res.tile([128, KA, BLOC], F32, tag="mask")
            nc.sync.dma_start(out=mask_sb, in_=maskD.rearrange("(c p) b -> p c b", p=128))
            brsv_sb = res.tile([128, KH], F32, tag="brsv")
            nc.sync.dma_start(out=brsv_sb, in_=biasRSV[:, :])
            bih_sb = res.tile([128, KM], F32, tag="bih")
            nc.sync.dma_start(out=bih_sb, in_=biasIH[:, :])
            bw_sb = res.tile([1, 1], F32, tag="bw")
            nc.sync.dma_start(out=bw_sb, in_=bw[:, :])
            # PS: proj_S + (b_S+b_R+b_V), [128, KH, BL] bf16
            ps_sb = res.tile([128, KH, BL], BF16, tag="ps")

            # ---------- precompute phase ----------
            with (
                tc.tile_pool(name="prew", bufs=1) as prew,
                tc.tile_pool(name="prehv", bufs=4) as prehv,
                tc.tile_pool(name="prestg", bufs=2) as prestg,
                tc.tile_pool(name="prepsum", bufs=4, space="PSUM") as prepsum,
            ):
                ws_sb = prew.tile([128, KH, H], BF16, tag="ws")
                nc.sync.dma_start(out=ws_sb, in_=WS_T.rearrange("(k p) m -> p k m", p=128))
                wv_sb = prew.tile([128, KH, H], BF16, tag="wv")
                nc.sync.dma_start(out=wv_sb, in_=WV_T.rearrange("(k p) m -> p k m", p=128))
                wihv_sb = prew.tile([128, KH, G4], BF16, tag="wihv")
                nc.sync.dma_start(out=wihv_sb, in_=WihV_T.rearrange("(k p) m -> p k m", p=128))
                wihs_sb = prew.tile([128, KH, G4], BF16, tag="wihs")
                nc.sync.dma_start(out=wihs_sb, in_=WihS_T.rearrange("(k p) m -> p k m", p=128))
                hsT_sb = prew.tile([128, KH, BL], BF16, tag="hsT")
                nc.sync.dma_start(out=hsT_sb, in_=hsT.rearrange("(k p) n -> p k n", p=128))

                # PS = W_S @ hsT + biasRSV
                for m in range(KH):
                    pps = prepsum.tile([128, BL], F32, tag="pp")
                    for kc in range(KH):
                        nc.tensor.matmul(
                            pps,
                            ws_sb[:, kc, m * 128:(m + 1) * 128],
                            hsT_sb[:, kc, :],
                            start=(kc == 0), stop=(kc == KH - 1),
                        )
                    nc.vector.tensor_scalar_add(ps_sb[:, m, :], pps, brsv_sb[:, m:m + 1])

                # PSg^T = h_s @ W_ihS^T in [(b,l), G4] layout: attention output
                # folded into the gate matmul (gates_S = PSg^T.T-style @ A).
                for c in range(KA):
                    for blk in range(KH):
                        ppg = prepsum.tile([128, 512], F32, tag="ppg")
                        for kc in range(KH):
                            nc.tensor.matmul(
                                ppg,
                                hsT_sb[:, kc, c * 128:(c + 1) * 128],
                                wihs_sb[:, kc, blk * 512:(blk + 1) * 512],
                                start=(kc == 0), stop=(kc == KH - 1),
                            )
                        nc.scalar.copy(psg_sb[:, c, blk * 512:(blk + 1) * 512], ppg)

                # PV / GV over hvT N-chunks
                for ncnk in range(n_nc):
                    nsl = slice(ncnk * NCW, (ncnk + 1) * NCW)
                    hv_t = []
                    for kc in range(KH):
                        t_ = prehv.tile([128, NCW], BF16, tag="hv")
                        nc.sync.dma_start(
                            out=t_, in_=hvT[kc * 128:(kc + 1) * 128, nsl])
                        hv_t.append(t_)
                    t0 = ncnk * NCW // BLOC
                    tw = NCW // BLOC
                    for m in range(KH):
                        ppv = prepsum.tile([128, NCW], F32, tag="pp")
                        for kc in range(KH):
                            nc.tensor.matmul(
                                ppv, wv_sb[:, kc, m * 128:(m + 1) * 128],
                                hv_t[kc], start=(kc == 0), stop=(kc == KH - 1))
                        stg = prestg.tile([128, NCW], F32, tag="pvstg")
                        nc.scalar.copy(stg, ppv)
                        nc.sync.dma_start(
                            out=PV_d[t0:t0 + tw, m, :, :].rearrange("t p b -> p t b"),
                            in_=stg.rearrange("p (t b) -> p t b", b=BLOC))
                    for m in range(KM):
                        pgv = prepsum.tile([128, NCW], F32, tag="pp")
                        for kc in range(KH):
                            nc.tensor.matmul(
                                pgv, wihv_sb[:, kc, m * 128:(m + 1) * 128],
                                hv_t[kc], start=(kc == 0), stop=(kc == KH - 1))
                        stg = prestg.tile([128, NCW], F32, tag="gvstg")
                        nc.vector.tensor_scalar_add(stg, pgv, bih_sb[:, m:m + 1])
                        nc.sync.dma_start(
                            out=GV_d[t0:t0 + tw, m, :, :].rearrange("t p b -> p t b"),
                            in_=stg.rearrange("p (t b) -> p t b", b=BLOC))

            # ---------- recurrence ----------
            psum = tc.alloc_tile_pool(name="psum", bufs=1, space="PSUM")
            czero = state.tile([128, 32], F32, tag="c")
            nc.vector.memset(czero, 0.0)
            hzero = res.tile([128, 32], BF16, tag="h0")
            nc.vector.memset(hzero, 0.0)
            c_prev = czero
            h_prev = hzero  # [128, (kc,b)] bf16

            HB = 8
            gv_cur = pv_cur = None
            hbuf = None

            for t in range(T):
                ic = t % Tc
                if ic == 0:
                    gv_cur = stream.tile([128, Tc, KM, BLOC], F32, tag="gv")
                    nc.sync.dma_start(
                        out=gv_cur,
                        in_=GV_d[t:t + Tc].rearrange("t m p b -> p t m b"))
                    pv_cur = stream.tile([128, Tc, KH, BLOC], F32, tag="pv")
                    nc.sync.dma_start(
                        out=pv_cur,
                        in_=PV_d[t:t + Tc].rearrange("t k p b -> p t k b"))
                ts_ = t % HB
                if ts_ == 0:
                    hbuf = houtp.tile([128, HB, KH, BLOC], BF16, tag="hb")

                # 1. proj_R -> psum_rv [128, (m,b)]
                psum_rv = psum.tile([128, KH, BLOC], F32, tag="rv")
                for m in range(KH):
                    for kc in range(KH):
                        nc.tensor.matmul(
                            psum_rv[:, m, :],
                            wr_sb[:, kc, m * 128:(m + 1) * 128],
                            h_prev[:, kc * BLOC:(kc + 1) * BLOC],
                            start=(kc == 0), stop=(kc == KH - 1))
                # 2. gates: W_hh @ h_prev part first (overlaps attention chain).
                # Single PSUM accumulation group for W_hh + PSg: only the very
                # first matmul carries start=True (zero-region clear); per-m
                # regions initialize via the has_written replace semantics.
                psum_gh = psum.tile([128, KM, BLOC], F32, tag="gh")
                for m in range(KM):
                    for kc in range(KH):
                        nc.tensor.matmul(
                            psum_gh[:, m, :],
                            whh_sb[:, kc, m * 128:(m + 1) * 128],
                            h_prev[:, kc * BLOC:(kc + 1) * BLOC],
                            start=(m == 0 and kc == 0), stop=False,
                            skip_group_check=True)
                # 3. rvt = proj_R + PV[t]  (bf16)
                rvt = work.tile([128, KH * BLOC], BF16, tag="rvt")
                nc.vector.tensor_tensor(
                    rvt.rearrange("p (k b) -> p k b", b=BLOC),
                    psum_rv, pv_cur[:, ic], ALU.add)
                # 4. e = tanh(PS + bcast_L(rvt)); beta accumulation
                e_all = work.tile([128, KH, BL], BF16, tag="e")
                psum_beta = psum.tile([1, BL], F32, tag="beta")
                for kc in range(KH):
                    sl = rvt[:, kc * BLOC:(kc + 1) * BLOC]
                    bc = bass.AP(tensor=sl.tensor, offset=sl.offset,
                                 ap=[sl.ap[0], [1, BLOC], [0, L]])
                    nc.vector.tensor_tensor(
                        e_all[:, kc].rearrange("p (b l) -> p b l", l=L),
                        ps_sb[:, kc].rearrange("p (b l) -> p b l", l=L),
                        bc, ALU.add)
                    nc.scalar.activation(e_all[:, kc], e_all[:, kc], AF.Tanh)
                    nc.tensor.matmul(
                        psum_beta, wvec_sb[:, kc:kc + 1], e_all[:, kc],
                        start=(kc == 0), stop=(kc == KH - 1))
                # 5. softmax pieces: expb (bf16), denom, rec
                expb = work.tile([1, BL], BF16, tag="expb")
                nc.scalar.activation(expb, psum_beta, AF.Exp, bias=bw_sb[:, 0:1])
                denom = work.tile([1, BLOC], F32, tag="denom")
                nc.vector.tensor_reduce(
                    denom, expb.rearrange("p (b l) -> p b l", l=L), AX.X, ALU.add)
                rec = work.tile([1, BLOC], F32, tag="rec")
                nc.vector.reciprocal(rec, denom)
                rec_bf = work.tile([1, BLOC], BF16, tag="recbf")
                nc.vector.tensor_copy(rec_bf, rec)
                # 6. alpha onto partitions: psum_A[c] = expb_chunk^T (x) rec
                psum_A = psum.tile([128, KA, BLOC], F32, tag="A")
                for c in range(KA):
                    nc.tensor.matmul(
                        psum_A[:, c, :],
                        expb[:, c * 128:(c + 1) * 128],
                        rec_bf, start=True, stop=True)
                A_sb = work.tile([128, KA, BLOC], BF16, tag="Asb")
                nc.vector.tensor_tensor(A_sb, psum_A, mask_sb, ALU.mult)
                # 7. gates_S = PSg^T @ A accumulated into the same PSUM group
                for m in range(KM):
                    for c in range(KA):
                        nc.tensor.matmul(
                            psum_gh[:, m, :],
                            psg_sb[:, c, m * 128:(m + 1) * 128],
                            A_sb[:, c, :],
                            start=False,
                            stop=(m == KM - 1 and c == KA - 1),
                            skip_group_check=True)
                # 8. gates = psum_gh + GV[t]; gate order [i, f, o, g]
                gates = work.tile([128, KM * BLOC], F32, tag="gates")
                nc.vector.tensor_tensor(
                    gates.rearrange("p (m b) -> p m b", b=BLOC),
                    psum_gh, gv_cur[:, ic], ALU.add)
                # 9. pointwise: th = tanh(gates) (i/f/o rows pre-scaled 0.5);
                # sigma = 0.5*th+0.5 via scalar Copy (no extra engine hop)
                th = work.tile([128, KM * BLOC], F32, tag="th")
                nc.scalar.activation(th, gates, AF.Tanh)
                sio = work.tile([128, 96], F32, tag="sio")
                nc.scalar.activation(sio, th[:, 0:96], AF.Copy, bias=0.5, scale=0.5)
                t1 = work.tile([128, 32], F32, tag="t1")
                nc.vector.tensor_tensor(t1, sio[:, 0:32], th[:, 96:128], ALU.mult)
                t2 = work.tile([128, 32], F32, tag="t2")
                nc.vector.tensor_tensor(t2, sio[:, 32:64], c_prev, ALU.mult)
                c_new = state.tile([128, 32], F32, tag="c")
                nc.vector.tensor_tensor(c_new, t1, t2, ALU.add)
                tc_ = work.tile([128, 32], F32, tag="tc")
                nc.scalar.activation(tc_, c_new, AF.Tanh)
                h_new = hbuf[:, ts_]  # [128, KH, BLOC] bf16 view
                nc.vector.tensor_tensor(
                    h_new.rearrange("p k b -> p (k b)"), sio[:, 64:96], tc_, ALU.mult)
                if debug and t == dbg_t:
                    nc.sync.dma_start(out=dbg["dbg_rvt"][:, :], in_=rvt)
                    nc.sync.dma_start(out=dbg["dbg_e"][:, :, :], in_=e_all)
                    nc.sync.dma_start(out=dbg["dbg_expb"][:, :], in_=expb)
                    nc.sync.dma_start(out=dbg["dbg_A"][:, :, :], in_=A_sb)
                    nc.sync.dma_start(out=dbg["dbg_gates"][:, :], in_=gates)
                    nc.sync.dma_start(out=dbg["dbg_th"][:, :], in_=th)
                    nc.sync.dma_start(out=dbg["dbg_c"][:, :], in_=c_new)
                c_prev = c_new
                h_prev = h_new.rearrange("p k b -> p (k b)")
                # 11. flush h ring
                if ts_ == HB - 1 or t == T - 1:
                    nb = ts_ + 1
                    t0 = t - nb + 1
                    nc.sync.dma_start(
                        out=out_c[t0:t0 + nb].rearrange("t k p b -> p (t k) b"),
                        in_=hbuf[:, :nb].rearrange("p t k b -> p (t k) b"))
            psum.release()
    nc.finalize()
    return nc


# ---------------- host side ----------------

def _gate_remap():
    """Row permutation + scale for gate order [i, f, o, g], i/f/o scaled 0.5."""
    idx = np.concatenate([
        np.arange(0, 512), np.arange(512, 1024),
        np.arange(1536, 2048), np.arange(1024, 1536)])
    scale = np.concatenate([
        np.full(1536, 0.5, np.float32), np.ones(512, np.float32)])
    return idx, scale


def prep_core_inputs(h_v, h_s, W, T=T_FULL):
    """Per-core input maps. W: dict of full weight arrays."""
    idx, gsc = _gate_remap()
    W_ih2 = W["W_ih"][idx] * gsc[:, None]
    W_hh2 = W["W_hh"][idx] * gsc[:, None]
    b2 = (W["b_ih"] + W["b_hh"])[idx] * gsc

    WS_T = np.ascontiguousarray(W["W_S"].T).astype(NP_BF16)
    WV_T = np.ascontiguousarray(W["W_V"].T).astype(NP_BF16)
    WihV_T = np.ascontiguousarray(W_ih2[:, :DV].T).astype(NP_BF16)
    WihS_T = np.ascontiguousarray(W_ih2[:, DV:].T).astype(NP_BF16)
    Whh_T = np.ascontiguousarray(W_hh2.T).astype(NP_BF16)
    WR_T = np.ascontiguousarray(W["W_R"].T).astype(NP_BF16)
    wvec = np.ascontiguousarray(W["W_w"][0][:, None]).astype(NP_BF16)
    biasRSV = np.ascontiguousarray(
        (W["b_S"] + W["b_R"] + W["b_V"]).reshape(KH, 128).T).astype(np.float32)
    biasIH = np.ascontiguousarray(b2.reshape(KM, 128).T).astype(np.float32)
    bw = np.ascontiguousarray(W["b_w"].reshape(1, 1)).astype(np.float32)
    maskD = np.zeros((BL, BLOC), np.float32)
    for j in range(BL):
        maskD[j, j // L] = 1.0
    maps = []
    for c in range(NCORES):
        bs = slice(c * BLOC, (c + 1) * BLOC)
        hvT = np.ascontiguousarray(
            h_v[bs, :T].transpose(2, 1, 0).reshape(DV, T * BLOC)).astype(NP_BF16)
        hsT = np.ascontiguousarray(
            h_s[bs].transpose(2, 0, 1).reshape(DS, BLOC * L)).astype(NP_BF16)
        maps.append({
            "hvT": hvT, "hsT": hsT,
            "WS_T": WS_T, "WV_T": WV_T,
            "WihV_T": WihV_T, "WihS_T": WihS_T, "Whh_T": Whh_T, "WR_T": WR_T,
            "wvec": wvec, "maskD": maskD,
            "biasRSV": biasRSV, "biasIH": biasIH, "bw": bw,
        })
    return maps


_NC_CACHE = {}


def kernel(**inputs):
    h_v = np.asarray(inputs["h_v"], dtype=np.float32)
    h_s = np.asarray(inputs["h_s"], dtype=np.float32)
    W = {k: np.asarray(v, dtype=np.float32) for k, v in inputs.items()}
    key = "full"
    if key not in _NC_CACHE:
        _NC_CACHE[key] = build_nc(T=T_FULL, Tc=16)
    nc = _NC_CACHE[key]
    maps = prep_core_inputs(h_v, h_s, W, T=T_FULL)
    res = run_bass_kernel_spmd(nc, maps, list(range(NCORES)))
    outs = []
    for c in range(NCORES):
        arr = np.asarray(res.results[c]["out_c"]).astype(np.float32)
        outs.append(np.ascontiguousarray(
            arr.transpose(3, 0, 1, 2).reshape(BLOC, T_FULL, H)))
    return np.concatenate(outs, axis=0).astype(np.float32)


if __name__ == "__main__":
    nc = build_nc(T=8, Tc=4)
    print("built ok")
